# revision 23
# baseline (speedup 1.0000x reference)
"""Trainium2 Bass kernel for MockBitNetLayer:

    scale = mean(|W|, axis=1)            # [O, 1]
    y = x @ (sign(W) * scale).T + bias   # [T, O]

Strategy (column-parallel over 8 NeuronCores), v7:
  - Each core owns an O/8 = 2048-column shard of W.T and bias; x is
    shared.  Host-side input marshaling (transpose + dtype cast + tile
    layout) is done in numpy during sharding; all model arithmetic
    (sign, |W| mean, matmul, scale/bias) runs on device.
  - Precision split over the contraction: the first NK8*128 rows of x
    in fp8e4 (consumed by DoubleRow MMs, 2 k-tiles per 216 ns slot),
    the rest in fp16 (1 k-tile per slot).  NK8=18 measures 1.99e-2
    against the fp32 reference (tolerance 2e-2); predicted exactly by
    a host-side numpy simulation of the quantization chain.
  - W arrives twice: k-major fp8e5 (sign path; values that would round
    to zero/denormal are host-fixed to +-2^-14 so sign() is exact) and
    o-major fp8e4 (scale path; mean|W| error ~0.06%, negligible).
  - Engine/queue split kills the v4 startup starvation (first MM was
    at 37 us, ~105 us of ramp losses):
      sync    : 64 W.T slab DMAs (nothing else -> no head-of-line)
      scalar  : 64 sign ops only
      vector  : |W| row reduces + 1/K -> then all psum evictions
                (tensor_scalar psum*scale+bias -> f16) + y DMAs
      gpsimd  : x chunk DMAs, bias, o-major W DMAs
      tensor  : nothing but the MM stream
  - Chunk 0 runs (ob0,ob1)/(ob2,ob3) interleaved across 8 PSUM banks,
    halving the sign-supply rate the ACT engine must sustain while the
    slab pipeline warms; later chunks use the rolling 4-bank pattern.
  - y is written as f16 (halves output traffic; +3e-4 error).
"""

import os
import sys

for _p in ("/opt/trn_rl_repo", "/root/.axon_site/_ro/trn_rl_repo"):
    if os.path.isdir(_p) and _p not in sys.path:
        sys.path.insert(0, _p)

import numpy as np
import ml_dtypes

import concourse.bacc as bacc
import concourse.mybir as mybir
import concourse.tile as tile
from concourse.bass import ds
from concourse.bass_utils import run_bass_kernel_spmd

P = 128
N_CORES = 8

T_FULL = 8192
K_FULL = 4096
O_FULL = 16384

NK8 = int(os.environ.get("NK8T", "18"))  # fp8 k-tiles (even, 0..32)
TCH = 512


def build_kernel_body(tc, xt8, xt16, wt, w8, b, yt, T, K, O, nk8):
    nc = tc.nc
    f32 = mybir.dt.float32
    f16 = mybir.dt.float16
    f8 = mybir.dt.float8e4
    f8w = mybir.dt.float8e5

    KT = K // P            # 32 k tiles
    KT16 = KT - nk8        # fp16 k tiles
    NPAIR = nk8 // 2       # fp8 DoubleRow pairs
    OT = O // P            # 16 o tiles
    NTCH = T // TCH        # token chunks
    OB = 4                 # o tiles per psum block (steady state)
    NOB = OT // OB
    # x16 arrives in three parts: 2 k-tiles (first MMs' dependency),
    # then the rest split evenly.
    XS = (2, (KT16 - 2 + 1) // 2, (KT16 - 2) // 2) if KT16 > 2 else (KT16, 0, 0)

    mult = mybir.AluOpType.mult
    addop = mybir.AluOpType.add

    with (
        tc.tile_pool(name="const", bufs=1) as const_pool,
        tc.tile_pool(name="wstage", bufs=3) as wstage,
        tc.tile_pool(name="astage", bufs=2) as astage,
        tc.tile_pool(name="swt", bufs=1) as swt_pool,
        tc.tile_pool(name="xt", bufs=3) as xt_pool,
        tc.tile_pool(name="out", bufs=8) as out_pool,
        tc.tile_pool(name="psum_mm", bufs=8, space="PSUM") as psum_mm,
    ):
        scale_sb = const_pool.tile([P, OT], f32)
        bias_sb = const_pool.tile([P, OT], f32)

        swt8 = swt_pool.tile([P, max(nk8, 1), O], f8)
        swt16 = swt_pool.tile([P, max(KT16, 1), O], f16)

        # ---- x chunks: gpsimd queue; x16 split so the first MMs' rows
        # land before the whole chunk does ----
        def load_x(c):
            parts = []
            k0 = 0
            for pi, nk in enumerate(XS):
                if nk == 0:
                    continue
                t = xt_pool.tile(
                    [P, nk, TCH], f16, tag=f"x16{pi}", name=f"x16{pi}_{c}"
                )
                nc.gpsimd.dma_start(t, xt16[c][:, ds(k0, nk), :])
                parts.append((k0, nk, t))
                k0 += nk
            t8 = None
            if nk8:
                t8 = xt_pool.tile([P, nk8, TCH], f8, tag="x8", name=f"x8_{c}")
                nc.gpsimd.dma_start(t8, xt8[c])
            return (parts, t8)

        def rhs16(xp, kt):
            for k0, nk, t in xp[0]:
                if kt < k0 + nk:
                    return t[:, kt - k0, :]
            raise AssertionError

        x_pre = {0: load_x(0)}

        # ---- sign path: k-major fp8e5 slabs -> ACT sign -> resident.
        # o-halves; fp16 k-tiles first within a half (MM consumption
        # order).  Per-DMA fixed cost (~1.5 us) dominated the v5 supply
        # rate, so slabs are fetched four k-tiles per DMA through the
        # 3-D [KT, P, O] view of W.T. ----
        OH = O // 2
        kt_order = list(range(nk8, KT)) + list(range(nk8))
        quads = []
        for seg in (kt_order[:KT16], kt_order[KT16:]):
            for j in range(0, len(seg), 4):
                quads.append(seg[j : j + 4])
        for oh in range(2):
            for q in quads:
                gs = len(q)
                slab = wstage.tile(
                    [P, gs, OH], f8w, tag=f"ws{gs}", name=f"ws{oh}_{q[0]}"
                )
                nc.sync.dma_start(
                    slab,
                    wt[ds(q[0], gs), :, ds(oh * OH, OH)].rearrange(
                        "g p o -> p g o"
                    ),
                )
                for gi, kt in enumerate(q):
                    if kt < nk8:
                        nc.scalar.sign(swt8[:, kt, ds(oh * OH, OH)], slab[:, gi, :])
                    else:
                        nc.scalar.sign(
                            swt16[:, kt - nk8, ds(oh * OH, OH)], slab[:, gi, :]
                        )

        # ---- scale path: o-major fp8e4 rows -> DVE |.| row mean.
        # These ride the sync ring behind the sign slabs: the gpsimd ring
        # is busy with x and only sustains ~80 GB/s, which made chunk 0's
        # second psum group wait ~7 us on scale_sb (v7 trace). ----
        for ot in range(OT):
            nc.sync.dma_start(
                bias_sb[:, ds(ot, 1)],
                b[ds(ot * P, P)].rearrange("(p one) -> p one", one=1),
            )
            wa = astage.tile([P, K], f8, tag="wa")
            nc.sync.dma_start(wa, w8[ds(ot * P, P), :])
            stot = const_pool.tile([P, 1], f32, tag="stot")
            nc.vector.tensor_reduce(
                out=stot,
                in_=wa,
                axis=mybir.AxisListType.X,
                op=addop,
                apply_absolute_value=True,
            )
            nc.vector.tensor_scalar_mul(scale_sb[:, ds(ot, 1)], stot, 1.0 / K)

        x_pre[1] = load_x(1)

        def evict(psum, ot, c, dmaq=None):
            out_sb = out_pool.tile([P, TCH], f16, name="osb")
            nc.vector.tensor_scalar(
                out_sb,
                psum,
                scale_sb[:, ds(ot, 1)],
                bias_sb[:, ds(ot, 1)],
                mult,
                addop,
            )
            (dmaq or nc.scalar).dma_start(
                yt[ds(ot * P, P), ds(c * TCH, TCH)], out_sb
            )

        def mm_group(psums, ots, xp):
            n_units = KT16 + NPAIR
            u = 0
            for kt in range(KT16):
                for psum, ot in zip(psums, ots):
                    nc.tensor.matmul(
                        psum,
                        lhsT=swt16[:, kt, ds(ot * P, P)],
                        rhs=rhs16(xp, kt),
                        start=(u == 0),
                        stop=(u == n_units - 1),
                    )
                u += 1
            for pr in range(NPAIR):
                for psum, ot in zip(psums, ots):
                    nc.tensor.matmul(
                        psum,
                        lhsT=swt8[:, ds(2 * pr, 2), ds(ot * P, P)],
                        rhs=xp[1][:, ds(2 * pr, 2), :],
                        start=(u == 0),
                        stop=(u == n_units - 1),
                        perf_mode=mybir.MatmulPerfMode.DoubleRow,
                    )
                u += 1

        # ---- main loop over token chunks ----
        for c in range(NTCH):
            xp = x_pre.pop(c)
            if c + 2 < NTCH:
                x_pre[c + 2] = load_x(c + 2)
            if c == 0:
                # ob-pairs across all 8 psum banks: halves the rate at
                # which fresh sign slabs are consumed during warmup.
                for obp in range(NOB // 2):
                    ots = list(range(obp * 2 * OB, (obp + 1) * 2 * OB))
                    psums = [
                        psum_mm.tile([P, TCH], f32, tag="acc", name=f"acc{i}")
                        for i in range(2 * OB)
                    ]
                    mm_group(psums, ots, xp)
                    for psum, ot in zip(psums, ots):
                        evict(psum, ot, c)
            else:
                if c == NTCH - 1:
                    # smaller final groups so the last evictions + y DMAs
                    # overlap the preceding MMs and shrink the tail
                    groups = [[0, 1, 2, 3], [4, 5, 6, 7], [8, 9, 10, 11],
                              [12, 13], [14, 15]]
                else:
                    groups = [
                        list(range(ob * OB, (ob + 1) * OB)) for ob in range(NOB)
                    ]
                for ots in groups:
                    psums = [
                        psum_mm.tile([P, TCH], f32, tag="acc", name=f"acc{i}")
                        for i in range(len(ots))
                    ]
                    mm_group(psums, ots, xp)
                    for i, (psum, ot) in enumerate(zip(psums, ots)):
                        dmaq = None
                        if c == NTCH - 1:
                            dmaq = nc.sync if i % 2 == 0 else nc.scalar
                        evict(psum, ot, c, dmaq)


def build_bass(T=T_FULL, K=K_FULL, O=O_FULL // N_CORES, nk8=NK8):
    nc = bacc.Bacc(trn_type="TRN2")
    f32 = mybir.dt.float32
    f16 = mybir.dt.float16
    f8 = mybir.dt.float8e4
    f8w = mybir.dt.float8e5
    KT16 = K // P - nk8
    NTCH = T // TCH
    xt8 = (
        nc.dram_tensor("xt8", [NTCH, P, nk8, TCH], f8, kind="ExternalInput").ap()
        if nk8
        else None
    )
    xt16 = (
        nc.dram_tensor("xt16", [NTCH, P, KT16, TCH], f16, kind="ExternalInput").ap()
        if KT16
        else None
    )
    wt = nc.dram_tensor("wt", [K // P, P, O], f8w, kind="ExternalInput").ap()
    w8 = nc.dram_tensor("w8", [O, K], f8, kind="ExternalInput").ap()
    b = nc.dram_tensor("b", [O], f32, kind="ExternalInput").ap()
    yt = nc.dram_tensor("yt", [O, T], f16, kind="ExternalOutput").ap()
    with tile.TileContext(nc) as tc:
        build_kernel_body(tc, xt8, xt16, wt, w8, b, yt, T, K, O, nk8)
    nc.finalize()
    return nc


_CACHED_NC = None


def _get_nc():
    global _CACHED_NC
    if _CACHED_NC is None:
        _CACHED_NC = build_bass()
    return _CACHED_NC


def make_in_maps(x, weight, bias):
    x = np.asarray(x, dtype=np.float32)
    weight = np.ascontiguousarray(np.asarray(weight, dtype=np.float32))
    bias = np.ascontiguousarray(np.asarray(bias, dtype=np.float32))
    O = weight.shape[0] // N_CORES
    K = x.shape[1]
    T = x.shape[0]
    KT16 = K // P - NK8
    # host-side marshaling: transpose + cast + tile layout
    xt = np.ascontiguousarray(x.T)  # [K, T]
    base = {}
    NTCH = T // TCH
    if NK8:
        base["xt8"] = np.ascontiguousarray(
            xt[: NK8 * P].reshape(NK8, P, NTCH, TCH).transpose(2, 1, 0, 3)
        ).astype(ml_dtypes.float8_e4m3fn)
    if KT16:
        base["xt16"] = np.ascontiguousarray(
            xt[NK8 * P :].reshape(KT16, P, NTCH, TCH).transpose(2, 1, 0, 3)
        ).astype(np.float16)
    # sign path: k-major fp8e5; values that would land in the denormal/
    # zero range are pinned to +-2^-14 (min normal) so sign() on device
    # is exactly sign(W).
    wtf = np.ascontiguousarray(weight.T)  # [K, O_FULL] f32
    wt8 = wtf.astype(ml_dtypes.float8_e5m2)
    tiny = np.abs(wtf) < np.float32(2.0**-14)
    if tiny.any():
        fix = np.copysign(np.float32(2.0**-14), wtf).astype(ml_dtypes.float8_e5m2)
        wt8 = np.where(tiny, fix, wt8)
    # scale path: o-major fp8e4
    w8 = weight.astype(ml_dtypes.float8_e4m3fn)  # [O_FULL, K]
    return [
        {
            **base,
            "wt": np.ascontiguousarray(wt8[:, c * O : (c + 1) * O]).reshape(
                K // P, P, O
            ),
            "w8": np.ascontiguousarray(w8[c * O : (c + 1) * O]),
            "b": bias[c * O : (c + 1) * O],
        }
        for c in range(N_CORES)
    ]


def kernel(x, weight, bias):
    nc = _get_nc()
    in_maps = make_in_maps(x, weight, bias)
    res = run_bass_kernel_spmd(nc, in_maps, list(range(N_CORES)))
    yt = np.concatenate([r["yt"] for r in res.results], axis=0)  # [O_FULL, T] f16
    return np.ascontiguousarray(yt.T.astype(np.float32))


# revision 25
# speedup vs baseline: 1.0121x; 1.0121x over previous
"""Trainium2 Bass kernel for MockBitNetLayer:

    scale = mean(|W|, axis=1)            # [O, 1]
    y = x @ (sign(W) * scale).T + bias   # [T, O]

Strategy (column-parallel over 8 NeuronCores), v7:
  - Each core owns an O/8 = 2048-column shard of W.T and bias; x is
    shared.  Host-side input marshaling (transpose + dtype cast + tile
    layout) is done in numpy during sharding; all model arithmetic
    (sign, |W| mean, matmul, scale/bias) runs on device.
  - Precision split over the contraction: the first NK8*128 rows of x
    in fp8e4 (consumed by DoubleRow MMs, 2 k-tiles per 216 ns slot),
    the rest in fp16 (1 k-tile per slot).  NK8=18 measures 1.99e-2
    against the fp32 reference (tolerance 2e-2); predicted exactly by
    a host-side numpy simulation of the quantization chain.
  - W arrives twice: k-major fp8e5 (sign path; values that would round
    to zero/denormal are host-fixed to +-2^-14 so sign() is exact) and
    o-major fp8e4 (scale path; mean|W| error ~0.06%, negligible).
  - Engine/queue split kills the v4 startup starvation (first MM was
    at 37 us, ~105 us of ramp losses):
      sync    : 64 W.T slab DMAs (nothing else -> no head-of-line)
      scalar  : 64 sign ops only
      vector  : |W| row reduces + 1/K -> then all psum evictions
                (tensor_scalar psum*scale+bias -> f16) + y DMAs
      gpsimd  : x chunk DMAs, bias, o-major W DMAs
      tensor  : nothing but the MM stream
  - Chunk 0 runs (ob0,ob1)/(ob2,ob3) interleaved across 8 PSUM banks,
    halving the sign-supply rate the ACT engine must sustain while the
    slab pipeline warms; later chunks use the rolling 4-bank pattern.
  - y is written as f16 (halves output traffic; +3e-4 error).
"""

import os
import sys

for _p in ("/opt/trn_rl_repo", "/root/.axon_site/_ro/trn_rl_repo"):
    if os.path.isdir(_p) and _p not in sys.path:
        sys.path.insert(0, _p)

import numpy as np
import ml_dtypes

import concourse.bacc as bacc
import concourse.mybir as mybir
import concourse.tile as tile
from concourse.bass import ds
from concourse.bass_utils import run_bass_kernel_spmd

P = 128
N_CORES = 8

T_FULL = 8192
K_FULL = 4096
O_FULL = 16384

NK8 = int(os.environ.get("NK8T", "18"))  # fp8 k-tiles (even, 0..32)
TCH = 512


def build_kernel_body(tc, xt8, xt16, wt, w8, b, yt, T, K, O, nk8):
    nc = tc.nc
    f32 = mybir.dt.float32
    f16 = mybir.dt.float16
    f8 = mybir.dt.float8e4
    f8w = mybir.dt.float8e5

    KT = K // P            # 32 k tiles
    KT16 = KT - nk8        # fp16 k tiles
    NPAIR = nk8 // 2       # fp8 DoubleRow pairs
    OT = O // P            # 16 o tiles
    NTCH = T // TCH        # token chunks
    OB = 4                 # o tiles per psum block (steady state)
    NOB = OT // OB
    # x16 arrives in three parts: 2 k-tiles (first MMs' dependency),
    # then the rest split evenly.
    XS = (2, (KT16 - 2 + 1) // 2, (KT16 - 2) // 2) if KT16 > 2 else (KT16, 0, 0)

    mult = mybir.AluOpType.mult
    addop = mybir.AluOpType.add

    with (
        tc.tile_pool(name="const", bufs=1) as const_pool,
        tc.tile_pool(name="wstage", bufs=3) as wstage,
        tc.tile_pool(name="astage", bufs=2) as astage,
        tc.tile_pool(name="swt", bufs=1) as swt_pool,
        tc.tile_pool(name="xt", bufs=3) as xt_pool,
        tc.tile_pool(name="out", bufs=8) as out_pool,
        tc.tile_pool(name="psum_mm", bufs=8, space="PSUM") as psum_mm,
    ):
        scale_sb = const_pool.tile([P, OT], f32)
        bias_sb = const_pool.tile([P, OT], f32)

        swt8 = swt_pool.tile([P, max(nk8, 1), O], f8)
        swt16 = swt_pool.tile([P, max(KT16, 1), O], f16)

        # ---- x chunks: gpsimd queue; x16 split so the first MMs' rows
        # land before the whole chunk does ----
        def load_x(c):
            parts = []
            k0 = 0
            for pi, nk in enumerate(XS):
                if nk == 0:
                    continue
                t = xt_pool.tile(
                    [P, nk, TCH], f16, tag=f"x16{pi}", name=f"x16{pi}_{c}"
                )
                nc.gpsimd.dma_start(t, xt16[c][:, ds(k0, nk), :])
                parts.append((k0, nk, t))
                k0 += nk
            t8 = None
            if nk8:
                t8 = xt_pool.tile([P, nk8, TCH], f8, tag="x8", name=f"x8_{c}")
                nc.gpsimd.dma_start(t8, xt8[c])
            return (parts, t8)

        def rhs16(xp, kt):
            for k0, nk, t in xp[0]:
                if kt < k0 + nk:
                    return t[:, kt - k0, :]
            raise AssertionError

        x_pre = {0: load_x(0)}

        # ---- sign path: k-major fp8e5 slabs -> ACT sign -> resident.
        # o-halves; fp16 k-tiles first within a half (MM consumption
        # order).  Per-DMA fixed cost (~1.5 us) dominated the v5 supply
        # rate, so slabs are fetched four k-tiles per DMA through the
        # 3-D [KT, P, O] view of W.T. ----
        OH = O // 2
        kt_order = list(range(nk8, KT)) + list(range(nk8))
        quads = []
        for seg in (kt_order[:KT16], kt_order[KT16:]):
            for j in range(0, len(seg), 4):
                quads.append(seg[j : j + 4])
        for oh in range(2):
            for q in quads:
                gs = len(q)
                slab = wstage.tile(
                    [P, gs, OH], f8w, tag=f"ws{gs}", name=f"ws{oh}_{q[0]}"
                )
                nc.sync.dma_start(
                    slab,
                    wt[ds(q[0], gs), :, ds(oh * OH, OH)].rearrange(
                        "g p o -> p g o"
                    ),
                )
                for gi, kt in enumerate(q):
                    if kt < nk8:
                        nc.scalar.sign(swt8[:, kt, ds(oh * OH, OH)], slab[:, gi, :])
                    else:
                        nc.scalar.sign(
                            swt16[:, kt - nk8, ds(oh * OH, OH)], slab[:, gi, :]
                        )

        # ---- scale path: o-major f16 rows -> DVE |.| row mean (2x DVE rate) ----
        for ot in range(OT):
            nc.gpsimd.dma_start(
                bias_sb[:, ds(ot, 1)],
                b[ds(ot * P, P)].rearrange("(p one) -> p one", one=1),
            )
            wa = astage.tile([P, K], f16, tag="wa")
            nc.gpsimd.dma_start(wa, w8[ds(ot * P, P), :])
            stot = const_pool.tile([P, 1], f32, tag="stot")
            nc.vector.tensor_reduce(
                out=stot,
                in_=wa,
                axis=mybir.AxisListType.X,
                op=addop,
                apply_absolute_value=True,
            )
            nc.vector.tensor_scalar_mul(scale_sb[:, ds(ot, 1)], stot, 1.0 / K)

        x_pre[1] = load_x(1)

        def evict(psum, ot, c, dmaq=None):
            out_sb = out_pool.tile([P, TCH], f16, name="osb")
            nc.vector.tensor_scalar(
                out_sb,
                psum,
                scale_sb[:, ds(ot, 1)],
                bias_sb[:, ds(ot, 1)],
                mult,
                addop,
            )
            (dmaq or nc.scalar).dma_start(
                yt[ds(ot * P, P), ds(c * TCH, TCH)], out_sb
            )

        def mm_group(psums, ots, xp):
            n_units = KT16 + NPAIR
            u = 0
            for kt in range(KT16):
                for psum, ot in zip(psums, ots):
                    nc.tensor.matmul(
                        psum,
                        lhsT=swt16[:, kt, ds(ot * P, P)],
                        rhs=rhs16(xp, kt),
                        start=(u == 0),
                        stop=(u == n_units - 1),
                    )
                u += 1
            for pr in range(NPAIR):
                for psum, ot in zip(psums, ots):
                    nc.tensor.matmul(
                        psum,
                        lhsT=swt8[:, ds(2 * pr, 2), ds(ot * P, P)],
                        rhs=xp[1][:, ds(2 * pr, 2), :],
                        start=(u == 0),
                        stop=(u == n_units - 1),
                        perf_mode=mybir.MatmulPerfMode.DoubleRow,
                    )
                u += 1

        # ---- main loop over token chunks ----
        for c in range(NTCH):
            xp = x_pre.pop(c)
            if c + 2 < NTCH:
                x_pre[c + 2] = load_x(c + 2)
            if c == 0:
                # ob-pairs across all 8 psum banks: halves the rate at
                # which fresh sign slabs are consumed during warmup.
                for obp in range(NOB // 2):
                    ots = list(range(obp * 2 * OB, (obp + 1) * 2 * OB))
                    psums = [
                        psum_mm.tile([P, TCH], f32, tag="acc", name=f"acc{i}")
                        for i in range(2 * OB)
                    ]
                    mm_group(psums, ots, xp)
                    for psum, ot in zip(psums, ots):
                        evict(psum, ot, c)
            else:
                if c == NTCH - 1:
                    # smaller final groups so the last evictions + y DMAs
                    # overlap the preceding MMs and shrink the tail
                    groups = [[0, 1, 2, 3], [4, 5, 6, 7], [8, 9, 10, 11],
                              [12, 13], [14, 15]]
                else:
                    groups = [
                        list(range(ob * OB, (ob + 1) * OB)) for ob in range(NOB)
                    ]
                for ots in groups:
                    psums = [
                        psum_mm.tile([P, TCH], f32, tag="acc", name=f"acc{i}")
                        for i in range(len(ots))
                    ]
                    mm_group(psums, ots, xp)
                    for i, (psum, ot) in enumerate(zip(psums, ots)):
                        dmaq = None
                        if c == NTCH - 1:
                            dmaq = nc.sync if i % 2 == 0 else nc.scalar
                        evict(psum, ot, c, dmaq)


def build_bass(T=T_FULL, K=K_FULL, O=O_FULL // N_CORES, nk8=NK8):
    nc = bacc.Bacc(trn_type="TRN2")
    f32 = mybir.dt.float32
    f16 = mybir.dt.float16
    f8 = mybir.dt.float8e4
    f8w = mybir.dt.float8e5
    KT16 = K // P - nk8
    NTCH = T // TCH
    xt8 = (
        nc.dram_tensor("xt8", [NTCH, P, nk8, TCH], f8, kind="ExternalInput").ap()
        if nk8
        else None
    )
    xt16 = (
        nc.dram_tensor("xt16", [NTCH, P, KT16, TCH], f16, kind="ExternalInput").ap()
        if KT16
        else None
    )
    wt = nc.dram_tensor("wt", [K // P, P, O], f8w, kind="ExternalInput").ap()
    w8 = nc.dram_tensor("w8", [O, K], f16, kind="ExternalInput").ap()
    b = nc.dram_tensor("b", [O], f32, kind="ExternalInput").ap()
    yt = nc.dram_tensor("yt", [O, T], f16, kind="ExternalOutput").ap()
    with tile.TileContext(nc) as tc:
        build_kernel_body(tc, xt8, xt16, wt, w8, b, yt, T, K, O, nk8)
    nc.finalize()
    return nc


_CACHED_NC = None


def _get_nc():
    global _CACHED_NC
    if _CACHED_NC is None:
        _CACHED_NC = build_bass()
    return _CACHED_NC


def make_in_maps(x, weight, bias):
    x = np.asarray(x, dtype=np.float32)
    weight = np.ascontiguousarray(np.asarray(weight, dtype=np.float32))
    bias = np.ascontiguousarray(np.asarray(bias, dtype=np.float32))
    O = weight.shape[0] // N_CORES
    K = x.shape[1]
    T = x.shape[0]
    KT16 = K // P - NK8
    # host-side marshaling: transpose + cast + tile layout
    xt = np.ascontiguousarray(x.T)  # [K, T]
    base = {}
    NTCH = T // TCH
    if NK8:
        base["xt8"] = np.ascontiguousarray(
            xt[: NK8 * P].reshape(NK8, P, NTCH, TCH).transpose(2, 1, 0, 3)
        ).astype(ml_dtypes.float8_e4m3fn)
    if KT16:
        base["xt16"] = np.ascontiguousarray(
            xt[NK8 * P :].reshape(KT16, P, NTCH, TCH).transpose(2, 1, 0, 3)
        ).astype(np.float16)
    # sign path: k-major fp8e5; values that would land in the denormal/
    # zero range are pinned to +-2^-14 (min normal) so sign() on device
    # is exactly sign(W).
    wtf = np.ascontiguousarray(weight.T)  # [K, O_FULL] f32
    wt8 = wtf.astype(ml_dtypes.float8_e5m2)
    tiny = np.abs(wtf) < np.float32(2.0**-14)
    if tiny.any():
        fix = np.copysign(np.float32(2.0**-14), wtf).astype(ml_dtypes.float8_e5m2)
        wt8 = np.where(tiny, fix, wt8)
    # scale path: o-major f16 (2x DVE reduce rate vs fp8)
    w8 = weight.astype(np.float16)  # [O_FULL, K]
    return [
        {
            **base,
            "wt": np.ascontiguousarray(wt8[:, c * O : (c + 1) * O]).reshape(
                K // P, P, O
            ),
            "w8": np.ascontiguousarray(w8[c * O : (c + 1) * O]),
            "b": bias[c * O : (c + 1) * O],
        }
        for c in range(N_CORES)
    ]


def kernel(x, weight, bias):
    nc = _get_nc()
    in_maps = make_in_maps(x, weight, bias)
    res = run_bass_kernel_spmd(nc, in_maps, list(range(N_CORES)))
    yt = np.concatenate([r["yt"] for r in res.results], axis=0)  # [O_FULL, T] f16
    return np.ascontiguousarray(yt.T.astype(np.float32))
